# revision 1
# baseline (speedup 1.0000x reference)
"""Trainium2 Bass kernel for nn_LCADecoderLayer (8-core SPMD, token-parallel).

Sharding: 4096 tokens split 512/core with balanced causal K/V (core c owns
batch0 rows [256c,256c+256) + batch1 rows [256(7-c),256(8-c)) so every
core's causal K/V context is exactly 2304 tokens). No collectives.

Device algorithm highlights:
- Everything runs in "transposed" activation layout where it kills
  transposes: q/k projections produce qT/kT directly; attention scores are
  computed transposed (scoresT[kv,q]) so softmax's kv-reduction is a PE
  ones-matmul and PV consumes expT directly (zero on-chip transposes in
  attention). Max-free softmax (scores bounded ~±10 for this input scale).
- LCA recurrence in transposed state wT[4096,512] with a@G factored as
  (a@W_lcaT)@W_lca - a*diag(G): no G build, no G storage; diag term folded
  in as an extra K-tile using diag(gs) built on device. Hardware For_i loop.
- All matmul operands bf16 (validated ~5e-3 relmax end to end), fp32 PSUM
  accumulation and fp32 state/softmax.
"""

from contextlib import ExitStack

import numpy as np
import ml_dtypes

import concourse.bass as bass
import concourse.mybir as mybir
import concourse.tile as tile
from concourse import bacc
from concourse.bass_utils import run_bass_kernel_spmd
from concourse.masks import make_identity

bf16 = ml_dtypes.bfloat16
F32, BF = mybir.dt.float32, mybir.dt.bfloat16
AF = mybir.ActivationFunctionType
OP = mybir.AluOpType

P = 128
B, S, D = 2, 2048, 2048
H, HD = 16, 128
DFF, DLCA = 8192, 4096
EPS, LAM = 1e-6, 0.1
NSTEPS = 10
ROPE_THETA = 10000.0

NCORE = 8
CHUNK = S // NCORE            # 256
TOK = 2 * CHUNK               # 512 own tokens / core
KV = S + CHUNK                # 2304 kv tokens / core
TB = TOK // P                 # 4
DB = D // P                   # 16
RB = DLCA // P                # 32
FB = DFF // P                 # 64
KVB = KV // P                 # 18
KVC = [512, 512, 512, 512, 256]   # kv free-dim chunks
ISQD = 1.0 / float(np.sqrt(HD))

UNROLL_LCA = False            # False -> tc.For_i hardware loop


# ----------------------------------------------------------------- host prep

def _core_token_map(c):
    b0 = np.arange(256 * c, 256 * c + 256)
    b1 = np.arange(256 * (7 - c), 256 * (8 - c))
    own = np.concatenate([b0, b1 + S])
    kv = np.concatenate([own, np.arange(0, 256 * c),
                         np.arange(0, 256 * (7 - c)) + S])
    return own, kv, kv % S, kv // S


def _rope_tables():
    inv_freq = 1.0 / (ROPE_THETA ** (np.arange(0, HD, 2, dtype=np.float32) / HD))
    t = np.arange(S, dtype=np.float32)
    freqs = np.outer(t, inv_freq)
    emb = np.concatenate([freqs, freqs], -1)           # [S, HD]
    return np.cos(emb).astype(np.float32), np.sin(emb).astype(np.float32)


def _per_head(w):   # [D, D] -> [H, D, HD] contiguous per head
    return np.ascontiguousarray(w.reshape(D, H, HD).transpose(1, 0, 2))


def _per_chunk(w, n):   # [D, X] -> [n, D, X/n]
    x = w.shape[1]
    return np.ascontiguousarray(w.reshape(w.shape[0], n, x // n).transpose(1, 0, 2))


# -------------------------------------------------------------- device build

def _dma_in(nc, pool, dram_ap, shape, dtype, tag=None, bufs_name=None):
    t = pool.tile(shape, dtype, tag=tag)
    nc.sync.dma_start(t[:], dram_ap)
    return t


def build_nc():
    nc = bacc.Bacc("TRN2", target_bir_lowering=False, debug=False,
                   num_devices=NCORE)

    def inp(name, shape, dt):
        return nc.dram_tensor(name, list(shape), dt, kind="ExternalInput").ap()

    x_kv = inp("x_kv", (KV, D), F32)
    xkvT = inp("xkvT", (D, KV), BF)
    x_own = inp("x_own", (TOK, D), F32)
    maskT = inp("maskT", (KV, TOK), BF)
    cosT = inp("cosT", (HD, KV), BF)
    sinT = inp("sinT", (HD, KV), BF)          # rows 0:64 pre-negated
    wq_r = inp("wq_r", (H, D, HD), BF)
    wk_r = inp("wk_r", (H, D, HD), BF)
    wv_g = inp("wv_g", (4, D, 512), BF)
    wo_n = inp("wo_n", (4, D, 512), BF)
    wlcan_r = inp("wlcan_r", (RB, D, P), BF)
    wlca_r = inp("wlca_r", (RB, D, P), BF)
    wlca_row = inp("wlca_row", (D, DLCA), BF)
    wlcats_d = inp("wlcats_d", (DB, DLCA, HD), BF)
    wlcats_n = inp("wlcats_n", (4, DLCA, 512), BF)
    wg_r = inp("wg_r", (FB, D, HD), BF)
    wu_r = inp("wu_r", (FB, D, HD), BF)
    wd_n = inp("wd_n", (4, DFF, 512), BF)
    y = nc.dram_tensor("y", [TOK, D], F32, kind="ExternalOutput").ap()

    with tile.TileContext(nc) as tc, ExitStack() as ctx:
        const = ctx.enter_context(tc.tile_pool(name="const", bufs=1))
        ident = const.tile([P, P], BF)
        make_identity(nc, ident)
        ones_col = const.tile([P, 1], BF)
        nc.vector.memset(ones_col[:], 1.0)
        ones_row = const.tile([1, P], F32)
        nc.vector.memset(ones_row[:], 1.0)
        bias_clam = const.tile([P, 1], F32)
        nc.vector.memset(bias_clam[:], -0.1 * LAM)
        bias_winit = const.tile([P, 1], F32)
        nc.vector.memset(bias_winit[:], -LAM)

        dram = ctx.enter_context(tc.tile_pool(name="dram", bufs=1, space="DRAM"))
        s_dram = dram.tile([KV, 1], F32)
        gs_dram = dram.tile([DLCA, 1], F32)

        # Lifetime-scoped resident pools (manually exited, alternating sides)
        hkp_cm = tc.tile_pool(name="hkp", bufs=1, side="left")
        hkp = hkp_cm.__enter__()
        hk = hkp.tile([P, DB, KV], BF)         # hkvT normed transposed, 73.7KB/p

        # ---------------- Phase A: rms scales s, hkvT ----------------
        with tc.tile_pool(name="pa", bufs=2) as pa:
            v_all = pa.tile([P, KVB], F32, tag="vall")
            for i in range(KVB):
                xt = pa.tile([P, D], F32, tag="xkv")
                nc.sync.dma_start(xt[:], x_kv[i * P:(i + 1) * P, :])
                sq = pa.tile([P, D], F32, tag="sq")
                nc.scalar.activation(sq[:], xt[:], AF.Square,
                                     accum_out=v_all[:, i:i + 1])
            t_all = pa.tile([P, KVB], F32, tag="tall")
            nc.vector.tensor_scalar(t_all[:], v_all[:], 1.0 / D, EPS,
                                    op0=OP.mult, op1=OP.add)
            r_all = pa.tile([P, KVB], F32, tag="rall")
            nc.vector.reciprocal(r_all[:], t_all[:])
            s_col = pa.tile([P, KVB], F32, tag="scol")
            nc.scalar.activation(s_col[:], r_all[:], AF.Sqrt)
            nc.sync.dma_start(
                s_dram[:].rearrange("(i p) one -> p (i one)", p=P), s_col[:])
            s_bc = pa.tile([P, KV], F32, tag="sbc")
            nc.sync.dma_start(
                s_bc[:], s_dram[:].rearrange("a b -> b a").broadcast_to((P, KV)))
            for j in range(DB):
                xT = pa.tile([P, KV], BF, tag="xT")
                nc.sync.dma_start(xT[:], xkvT[j * P:(j + 1) * P, :])
                nc.vector.tensor_tensor(hk[:, j, :], xT[:], s_bc[:], op=OP.mult)

        # ---------------- Phase B: attention ----------------
        attp_cm = tc.tile_pool(name="attp", bufs=1, side="right")
        attp = attp_cm.__enter__()
        attnT = attp.tile([P, DB, TOK], BF)

        with (
            tc.tile_pool(name="pb", bufs=1) as pb,
            tc.tile_pool(name="pbs1", bufs=1) as pbs1,
            tc.tile_pool(name="pbs2", bufs=2) as pbs2,
            tc.tile_pool(name="pbps", bufs=2, space="PSUM") as pbps,
        ):
            mk = pb.tile([P, KVB, TOK], BF)
            nc.sync.dma_start(mk[:], maskT.rearrange("(i p) t -> p i t", p=P))
            cos_sb = pb.tile([P, KV], BF)
            nc.sync.dma_start(cos_sb[:], cosT[:])
            sin_sb = pb.tile([P, KV], BF)
            nc.sync.dma_start(sin_sb[:], sinT[:])
            expT = pb.tile([P, KVB, TOK], BF)

            def rope_evict(dst, ps, n0, nsz):
                qc = pbs1.tile([P, 512], F32, tag="rope_c", name="qc")
                nc.vector.tensor_tensor(qc[:, :nsz], ps[:, :nsz],
                                        cos_sb[:, n0:n0 + nsz], op=OP.mult)
                qr = pbs1.tile([P, 512], F32, tag="rope_r", name="qr")
                hh2 = HD // 2
                nc.vector.tensor_tensor(qr[:hh2, :nsz], ps[hh2:, :nsz],
                                        sin_sb[:hh2, n0:n0 + nsz], op=OP.mult)
                nc.vector.tensor_tensor(qr[hh2:, :nsz], ps[:hh2, :nsz],
                                        sin_sb[hh2:, n0:n0 + nsz], op=OP.mult)
                nc.vector.tensor_tensor(dst, qc[:, :nsz], qr[:, :nsz], op=OP.add)

            for g in range(4):
                vg = pb.tile([P, KVB, 512], BF, tag="vg", name="vg")
                wv_sb = pbs1.tile([P, DB, 512], BF, tag="wv", name="wv_sb")
                nc.sync.dma_start(
                    wv_sb[:], wv_g[g].rearrange("(j p) n -> p j n", p=P))
                for t in range(KVB):
                    ps_v = pbps.tile([P, 512], F32, tag="ps_a", name="ps_v")
                    for j in range(DB):
                        nc.tensor.matmul(ps_v[:], hk[:, j, t * P:(t + 1) * P],
                                         wv_sb[:, j, :], start=(j == 0),
                                         stop=(j == DB - 1))
                    nc.scalar.copy(vg[:, t, :], ps_v[:])

                for h4 in range(4):
                    hh = g * 4 + h4
                    wq_sb = pbs2.tile([P, DB, HD], BF, tag="wq", name="wq_sb")
                    nc.sync.dma_start(
                        wq_sb[:], wq_r[hh].rearrange("(j p) e -> p j e", p=P))
                    wk_sb = pbs2.tile([P, DB, HD], BF, tag="wk", name="wk_sb")
                    nc.sync.dma_start(
                        wk_sb[:], wk_r[hh].rearrange("(j p) e -> p j e", p=P))

                    qT = pbs2.tile([P, TOK], BF, tag="qT", name="qT")
                    ps_q = pbps.tile([P, 512], F32, tag="ps_a", name="ps_q")
                    for j in range(DB):
                        nc.tensor.matmul(ps_q[:], wq_sb[:, j, :],
                                         hk[:, j, :TOK], start=(j == 0),
                                         stop=(j == DB - 1))
                    rope_evict(qT[:], ps_q, 0, TOK)

                    kT = pbs2.tile([P, KV], BF, tag="kT", name="kT")
                    n0 = 0
                    for nsz in KVC:
                        ps_k = pbps.tile([P, 512], F32, tag="ps_a", name="ps_k")
                        for j in range(DB):
                            nc.tensor.matmul(ps_k[:, :nsz], wk_sb[:, j, :],
                                             hk[:, j, n0:n0 + nsz],
                                             start=(j == 0), stop=(j == DB - 1))
                        rope_evict(kT[:, n0:n0 + nsz], ps_k, n0, nsz)
                        n0 += nsz

                    # scoresT -> exp (max-free softmax)
                    for t in range(KVB):
                        ps_s = pbps.tile([P, TOK], F32, tag="ps_s", name="ps_s")
                        nc.tensor.matmul(ps_s[:], kT[:, t * P:(t + 1) * P],
                                         qT[:], start=True, stop=True)
                        msc = pbs1.tile([P, TOK], F32, tag="msc", name="msc")
                        nc.vector.tensor_tensor(msc[:], ps_s[:], mk[:, t, :],
                                                op=OP.add)
                        nc.scalar.activation(expT[:, t, :], msc[:], AF.Exp,
                                             scale=ISQD)
                    ps_sum = pbps.tile([1, TOK], F32, tag="ps_sum",
                                       name="ps_sum")
                    for t in range(KVB):
                        nc.tensor.matmul(ps_sum[:], ones_col[:], expT[:, t, :],
                                         start=(t == 0), stop=(t == KVB - 1))
                    r_row = pbs1.tile([1, TOK], F32, tag="r_row", name="r_row")
                    nc.vector.reciprocal(r_row[:], ps_sum[:])
                    ps_rbc = pbps.tile([P, TOK], F32, tag="ps_s", name="ps_rbc")
                    nc.tensor.matmul(ps_rbc[:], ones_row[:], r_row[:],
                                     start=True, stop=True)
                    r_bc = pbs1.tile([P, TOK], F32, tag="r_bc", name="r_bc")
                    nc.scalar.copy(r_bc[:], ps_rbc[:])
                    ps_pv = pbps.tile([P, TOK], F32, tag="ps_pv", name="ps_pv")
                    for t in range(KVB):
                        nc.tensor.matmul(ps_pv[:], vg[:, t, h4 * P:(h4 + 1) * P],
                                         expT[:, t, :], start=(t == 0),
                                         stop=(t == KVB - 1))
                    nc.vector.tensor_tensor(attnT[:, hh, :], ps_pv[:], r_bc[:],
                                            op=OP.mult)

        hkp_cm.__exit__(None, None, None)
        h1p_cm = tc.tile_pool(name="h1p", bufs=1, side="left")
        h1p = h1p_cm.__enter__()
        h1 = h1p.tile([P, TB, D], F32)

        # ---------------- Phase C: attn @ Wo + residual ----------------
        with (
            tc.tile_pool(name="pc", bufs=2) as pc,
            tc.tile_pool(name="pcps", bufs=2, space="PSUM") as pcps,
        ):
            for n in range(4):
                wo_sb = pc.tile([P, DB, 512], BF, tag="wo", name="wo_sb")
                nc.sync.dma_start(
                    wo_sb[:], wo_n[n].rearrange("(j p) c -> p j c", p=P))
                for m in range(TB):
                    ps_o = pcps.tile([P, 512], F32, tag="ps_o", name="ps_o")
                    for k in range(DB):
                        nc.tensor.matmul(ps_o[:], attnT[:, k, m * P:(m + 1) * P],
                                         wo_sb[:, k, :], start=(k == 0),
                                         stop=(k == DB - 1))
                    xo = pc.tile([P, 512], F32, tag="xo", name="xo")
                    nc.sync.dma_start(
                        xo[:], x_own[m * P:(m + 1) * P, n * 512:(n + 1) * 512])
                    nc.vector.tensor_tensor(h1[:, m, n * 512:(n + 1) * 512],
                                            ps_o[:], xo[:], op=OP.add)

        attp_cm.__exit__(None, None, None)
        hnp_cm = tc.tile_pool(name="hnp", bufs=1, side="right")
        hnp = hnp_cm.__enter__()
        hnT = hnp.tile([P, DB, TOK], BF)

        # ------------- Phase D1: hnT (rmsnorm of h1, transposed) + gs -------------
        with (
            tc.tile_pool(name="pd1s", bufs=1) as pd1s,
            tc.tile_pool(name="pdps", bufs=2, space="PSUM") as pdps,
        ):
            for m in range(TB):
                sq = pd1s.tile([P, D], F32, tag="sq2", name="sq")
                v2 = pd1s.tile([P, 1], F32, tag="v2", name="v2")
                nc.scalar.activation(sq[:], h1[:, m, :], AF.Square,
                                     accum_out=v2[:])
                t2 = pd1s.tile([P, 1], F32, tag="t2", name="t2")
                nc.vector.tensor_scalar(t2[:], v2[:], 1.0 / D, EPS,
                                        op0=OP.mult, op1=OP.add)
                r2 = pd1s.tile([P, 1], F32, tag="r2", name="r2")
                nc.vector.reciprocal(r2[:], t2[:])
                s2 = pd1s.tile([P, 1], F32, tag="s2", name="s2")
                nc.scalar.activation(s2[:], r2[:], AF.Sqrt)
                hn = pd1s.tile([P, D], BF, tag="hn", name="hn")
                nc.vector.tensor_scalar(hn[:], h1[:, m, :], s2[:], None,
                                        op0=OP.mult)
                for j in range(DB):
                    ps_t = pdps.tile([P, P], BF, tag="ps_tr", name="ps_t")
                    nc.tensor.transpose(ps_t[:], hn[:, j * P:(j + 1) * P],
                                        ident[:])
                    nc.scalar.copy(hnT[:, j, m * P:(m + 1) * P], ps_t[:])

        with (
            tc.tile_pool(name="pd2s", bufs=2) as pd2s,
            tc.tile_pool(name="pdgs", bufs=1, space="PSUM") as pdgs,
        ):
            # gs = diag(W^T W): ones-matmul over squared W row-blocks
            for j in range(DB):
                wl_sb = pd2s.tile([P, DLCA], BF, tag="wlrow", name="wl_sb")
                nc.sync.dma_start(wl_sb[:], wlca_row[j * P:(j + 1) * P, :])
                w2 = pd2s.tile([P, DLCA], BF, tag="w2", name="w2")
                nc.vector.tensor_tensor(w2[:], wl_sb[:], wl_sb[:], op=OP.mult)
                for cix in range(8):
                    ps_gs = pdgs.tile([1, 512], F32, tag=f"ps_gs{cix}",
                                      name=f"ps_gs{cix}")
                    nc.tensor.matmul(ps_gs[:], ones_col[:],
                                     w2[:, cix * 512:(cix + 1) * 512],
                                     start=(j == 0), stop=(j == DB - 1))
                    if j == DB - 1:
                        grow = pd2s.tile([1, 512], F32, tag="grow", name="grow")
                        nc.scalar.copy(grow[:], ps_gs[:])
                        nc.sync.dma_start(
                            gs_dram[cix * 512:(cix + 1) * 512, :]
                            .rearrange("a b -> b a"), grow[:])

        h1p_cm.__exit__(None, None, None)
        wcp_cm = tc.tile_pool(name="wcp", bufs=1, side="left")
        wcp = wcp_cm.__enter__()
        wT = wcp.tile([P, RB, TOK], F32)       # 64KB/p
        clamT = wcp.tile([P, RB, TOK], BF)     # 32KB/p
        diag_gs = wcp.tile([P, RB, P], BF)     # 8KB/p

        # ------------- Phase D2: clamT + wT init + diag_gs -------------
        with (
            tc.tile_pool(name="pd3s", bufs=2) as pd3s,
            tc.tile_pool(name="pd3ps", bufs=2, space="PSUM") as pd3ps,
        ):
            for r in range(RB):
                wn_sb = pd3s.tile([P, DB, P], BF, tag="wn", name="wn_sb")
                nc.sync.dma_start(
                    wn_sb[:], wlcan_r[r].rearrange("(j p) c -> p j c", p=P))
                ps_b = pd3ps.tile([P, TOK], F32, tag="ps_b", name="ps_b")
                for j in range(DB):
                    nc.tensor.matmul(ps_b[:], wn_sb[:, j, :], hnT[:, j, :],
                                     start=(j == 0), stop=(j == DB - 1))
                nc.scalar.activation(clamT[:, r, :], ps_b[:], AF.Identity,
                                     scale=0.1, bias=bias_clam[:])
                nc.scalar.activation(wT[:, r, :], ps_b[:], AF.Identity,
                                     scale=0.1, bias=bias_winit[:])
            gst = pd3s.tile([P, RB], F32, tag="gst", name="gst")
            nc.sync.dma_start(
                gst[:], gs_dram[:].rearrange("(r p) one -> p (r one)", p=P))
            for r in range(RB):
                nc.vector.tensor_scalar(diag_gs[:, r, :], ident[:],
                                        gst[:, r:r + 1], 0.1,
                                        op0=OP.mult, op1=OP.mult)

        hnp_cm.__exit__(None, None, None)
        atp_cm = tc.tile_pool(name="atp", bufs=1, side="right")
        atp = atp_cm.__enter__()
        aT = atp.tile([P, RB, TOK], BF)

        # ---------------- Phase E: LCA recurrence ----------------
        with (
            tc.tile_pool(name="pe", bufs=2) as pe,
            tc.tile_pool(name="pe1", bufs=1) as pe1,
            tc.tile_pool(name="peps", bufs=2, space="PSUM") as peps,
        ):
            def lca_step():
                yT = pe1.tile([P, DB, TOK], BF, tag="yT", name="yT")
                for r in range(RB):
                    nc.scalar.activation(aT[:, r, :], wT[:, r, :], AF.Relu)
                for d in range(DB):
                    w1_sb = pe.tile([P, RB, P], BF, tag="w1", name="w1_sb")
                    nc.sync.dma_start(
                        w1_sb[:], wlcats_d[d].rearrange("(k p) c -> p k c", p=P))
                    ps_y = peps.tile([P, TOK], F32, tag="ps_y", name="ps_y")
                    for k in range(RB):
                        nc.tensor.matmul(ps_y[:], w1_sb[:, k, :], aT[:, k, :],
                                         start=(k == 0), stop=(k == RB - 1))
                    nc.scalar.copy(yT[:, d, :], ps_y[:])
                for r in range(RB):
                    w2_sb = pe.tile([P, DB, P], BF, tag="w2s", name="w2_sb")
                    nc.sync.dma_start(
                        w2_sb[:], wlca_r[r].rearrange("(j p) c -> p j c", p=P))
                    ps_z = peps.tile([P, TOK], F32, tag="ps_z", name="ps_z")
                    for j in range(DB):
                        nc.tensor.matmul(ps_z[:], w2_sb[:, j, :], yT[:, j, :],
                                         start=(j == 0), stop=False)
                    nc.tensor.matmul(ps_z[:], diag_gs[:, r, :], aT[:, r, :],
                                     start=False, stop=True)
                    u1 = pe.tile([P, TOK], F32, tag="u1", name="u1")
                    nc.vector.tensor_tensor(u1[:], ps_z[:], clamT[:, r, :],
                                            op=OP.add)
                    w9 = pe.tile([P, TOK], F32, tag="w9", name="w9")
                    nc.scalar.activation(w9[:], wT[:, r, :], AF.Identity,
                                         scale=0.9)
                    nc.vector.tensor_tensor(wT[:, r, :], w9[:], u1[:], op=OP.add)

            if UNROLL_LCA:
                for _ in range(NSTEPS - 1):
                    lca_step()
            else:
                with tc.For_i(0, NSTEPS - 1, 1):
                    lca_step()
            for r in range(RB):
                nc.scalar.activation(aT[:, r, :], wT[:, r, :], AF.Relu)

        wcp_cm.__exit__(None, None, None)
        h2p_cm = tc.tile_pool(name="h2p", bufs=1, side="left")
        h2p = h2p_cm.__enter__()
        h2 = h2p.tile([P, TB, D], F32)

        # ---------------- Phase F: h2 = a @ W_lca^T ----------------
        with (
            tc.tile_pool(name="pf", bufs=2) as pf,
            tc.tile_pool(name="pfps", bufs=2, space="PSUM") as pfps,
        ):
            for n in range(4):
                wt_sb = pf.tile([P, RB, 512], BF, tag="wts", name="wt_sb")
                nc.sync.dma_start(
                    wt_sb[:], wlcats_n[n].rearrange("(k p) c -> p k c", p=P))
                for m in range(TB):
                    ps_h = pfps.tile([P, 512], F32, tag="ps_h", name="ps_h")
                    for k in range(RB):
                        nc.tensor.matmul(ps_h[:], aT[:, k, m * P:(m + 1) * P],
                                         wt_sb[:, k, :], start=(k == 0),
                                         stop=(k == RB - 1))
                    nc.scalar.activation(h2[:, m, n * 512:(n + 1) * 512],
                                         ps_h[:], AF.Identity, scale=-10.0)

        atp_cm.__exit__(None, None, None)

        # ---------------- Phase G: MLP ----------------
        with (
            tc.tile_pool(name="pg", bufs=1, side="right") as pg,
            tc.tile_pool(name="pgs1", bufs=1) as pgs1,
            tc.tile_pool(name="pgs", bufs=2) as pgs,
            tc.tile_pool(name="pgps", bufs=2, space="PSUM") as pgps,
            tc.tile_pool(name="pgpd", bufs=1, space="PSUM") as pgpd,
        ):
            prodT = pg.tile([P, FB, TOK], BF)      # 64KB/p
            mT = pg.tile([P, DB, TOK], BF)
            for m in range(TB):
                sq = pgs1.tile([P, D], F32, tag="sq3", name="sq")
                v3 = pgs1.tile([P, 1], F32, tag="v3", name="v3")
                nc.scalar.activation(sq[:], h2[:, m, :], AF.Square,
                                     accum_out=v3[:])
                t3 = pgs1.tile([P, 1], F32, tag="t3", name="t3")
                nc.vector.tensor_scalar(t3[:], v3[:], 1.0 / D, EPS,
                                        op0=OP.mult, op1=OP.add)
                r3 = pgs1.tile([P, 1], F32, tag="r3", name="r3")
                nc.vector.reciprocal(r3[:], t3[:])
                s3 = pgs1.tile([P, 1], F32, tag="s3", name="s3")
                nc.scalar.activation(s3[:], r3[:], AF.Sqrt)
                mb = pgs1.tile([P, D], BF, tag="mb", name="mb")
                nc.vector.tensor_scalar(mb[:], h2[:, m, :], s3[:], None,
                                        op0=OP.mult)
                for j in range(DB):
                    ps_t = pgps.tile([P, P], BF, tag="ps_tr3", name="ps_t")
                    nc.tensor.transpose(ps_t[:], mb[:, j * P:(j + 1) * P],
                                        ident[:])
                    nc.scalar.copy(mT[:, j, m * P:(m + 1) * P], ps_t[:])

            for f in range(FB):
                wgs = pgs.tile([P, DB, HD], BF, tag="wgs", name="wgs")
                nc.sync.dma_start(
                    wgs[:], wg_r[f].rearrange("(j p) e -> p j e", p=P))
                ps_g = pgps.tile([P, TOK], F32, tag="ps_g", name="ps_g")
                for j in range(DB):
                    nc.tensor.matmul(ps_g[:], wgs[:, j, :], mT[:, j, :],
                                     start=(j == 0), stop=(j == DB - 1))
                gT = pgs.tile([P, TOK], BF, tag="gT", name="gT")
                nc.scalar.activation(gT[:], ps_g[:], AF.Silu)
                wus = pgs.tile([P, DB, HD], BF, tag="wus", name="wus")
                nc.sync.dma_start(
                    wus[:], wu_r[f].rearrange("(j p) e -> p j e", p=P))
                ps_u = pgps.tile([P, TOK], F32, tag="ps_g", name="ps_u")
                for j in range(DB):
                    nc.tensor.matmul(ps_u[:], wus[:, j, :], mT[:, j, :],
                                     start=(j == 0), stop=(j == DB - 1))
                nc.vector.tensor_tensor(prodT[:, f, :], ps_u[:], gT[:],
                                        op=OP.mult)

            for n in range(4):
                ps_d = [pgpd.tile([P, 512], F32, tag=f"ps_d{m}",
                                  name=f"ps_d{m}")
                        for m in range(TB)]
                for kg in range(8):
                    wds = pgs.tile([P, 8, 512], BF, tag="wds", name="wds")
                    nc.sync.dma_start(
                        wds[:], wd_n[n][kg * 1024:(kg + 1) * 1024, :]
                        .rearrange("(k p) c -> p k c", p=P))
                    for m in range(TB):
                        for k in range(8):
                            kk = kg * 8 + k
                            nc.tensor.matmul(
                                ps_d[m][:], prodT[:, kk, m * P:(m + 1) * P],
                                wds[:, k, :], start=(kg == 0 and k == 0),
                                stop=(kg == 7 and k == 7))
                for m in range(TB):
                    yo = pgs.tile([P, 512], F32, tag="yo", name="yo")
                    nc.vector.tensor_tensor(yo[:], ps_d[m][:],
                                            h2[:, m, n * 512:(n + 1) * 512],
                                            op=OP.add)
                    nc.sync.dma_start(
                        y[m * P:(m + 1) * P, n * 512:(n + 1) * 512], yo[:])

        h2p_cm.__exit__(None, None, None)

    nc.compile()
    return nc


_NC_CACHE = None


def _get_nc():
    global _NC_CACHE
    if _NC_CACHE is None:
        _NC_CACHE = build_nc()
    return _NC_CACHE


def _prep_weights(inputs):
    f32 = np.float32
    wln_in = np.asarray(inputs["w_ln_in"], f32)
    wln_lca = np.asarray(inputs["w_ln_lca"], f32)
    wln_post = np.asarray(inputs["w_ln_post"], f32)
    Wq = np.asarray(inputs["Wq"], f32) * wln_in[:, None]
    Wk = np.asarray(inputs["Wk"], f32) * wln_in[:, None]
    Wv = np.asarray(inputs["Wv"], f32) * wln_in[:, None]
    Wo = np.asarray(inputs["Wo"], f32)
    Wlca = np.asarray(inputs["W_lca"], f32)
    Wlca_n = Wlca * wln_lca[:, None]
    WlcaT_s = np.ascontiguousarray(-0.1 * Wlca.T)
    Wg = np.asarray(inputs["W_gate"], f32) * wln_post[:, None]
    Wu = np.asarray(inputs["W_up"], f32) * wln_post[:, None]
    Wd = np.asarray(inputs["W_down"], f32)
    c = lambda a: np.ascontiguousarray(a).astype(bf16)
    return {
        "wq_r": c(_per_head(Wq)), "wk_r": c(_per_head(Wk)),
        "wv_g": c(_per_chunk(Wv, 4)), "wo_n": c(_per_chunk(Wo, 4)),
        "wlcan_r": c(_per_chunk(Wlca_n, RB)),
        "wlca_r": c(_per_chunk(Wlca, RB)),
        "wlca_row": c(Wlca),
        "wlcats_d": c(_per_chunk(WlcaT_s, DB)),
        "wlcats_n": c(_per_chunk(WlcaT_s, 4)),
        "wg_r": c(_per_chunk(Wg, FB)), "wu_r": c(_per_chunk(Wu, FB)),
        "wd_n": c(_per_chunk(Wd, 4)),
    }


def make_in_maps(inputs):
    hs = np.asarray(inputs["hidden_states"], np.float32).reshape(B * S, D)
    wmaps = _prep_weights(inputs)
    cos, sin = _rope_tables()
    in_maps, owns = [], []
    for cix in range(NCORE):
        own, kv, kv_pos, kv_batch = _core_token_map(cix)
        x_kv = np.ascontiguousarray(hs[kv])
        xkvT = np.ascontiguousarray(x_kv.T).astype(bf16)
        q_pos, q_batch = own % S, own // S
        vis = (kv_batch[:, None] == q_batch[None, :]) & (
            kv_pos[:, None] <= q_pos[None, :])
        maskT = np.where(vis, 0.0, -1e30).astype(np.float32).astype(bf16)
        cosT = np.ascontiguousarray(cos[kv_pos].T).astype(bf16)
        sinT = np.ascontiguousarray(sin[kv_pos].T)
        sinT[:HD // 2] *= -1.0
        sinT = sinT.astype(bf16)
        m = {
            "x_kv": x_kv, "xkvT": xkvT,
            "x_own": np.ascontiguousarray(hs[own]),
            "maskT": maskT, "cosT": cosT, "sinT": sinT, **wmaps,
        }
        in_maps.append(m)
        owns.append(own)
    return in_maps, owns


def kernel(**inputs) -> np.ndarray:
    nc = _get_nc()
    in_maps, owns = make_in_maps(inputs)
    res = run_bass_kernel_spmd(nc, in_maps, core_ids=list(range(NCORE)))
    out = np.zeros((B * S, D), np.float32)
    for cix in range(NCORE):
        out[owns[cix]] = res.results[cix]["y"]
    return out.reshape(B, S, D)

